# revision 1
# baseline (speedup 1.0000x reference)
"""AllAtomFAPE loss kernel for Trainium2 (8 NeuronCores, SPMD).

Problem: b=1, N=384 res, F=8 frames/res -> NF=3072 frames; A=14 atoms/res
-> NA=5376 atoms. Output: scalar (shape (1,)) masked clamped FAPE.

Algorithm (factorized pairwise distance):
  With P = pR pR^T, T = tR tR^T, M = pR tR^T (per frame, 3x3),
    d2(f,a) = (pp-pt)^T P (pp-pt) + (tp-tt)^T T (tp-tt)
              - 2 (pp-pt)^T M (tp-tt)
  expands into a K=34 dot product between a frame feature vector W[:,f]
  and an atom feature vector Z[:,a]:
    rows 0-8   : P[i,j]            <->  pp_i pp_j
    rows 9-17  : T[i,j]            <->  tp_i tp_j
    rows 18-26 : M[i,j]            <->  -2 pp_i tp_j
    rows 27-29 : 2(M tt - P pt)    <->  pp
    rows 30-32 : 2(M^T pt - T tt)  <->  tp
    row  33    : c_f               <->  1
  so the whole pairwise computation is one (34 x NF) x (34 x NA) matmul
  on the TensorEngine (bf16). Then (ScalarE) d = m_a*sqrt(d2+eps) via
  sqrt(scale*x+bias) with per-partition scale=m^2, bias=m^2*eps, and
  (VectorE) fused clamp+reduce: tensor_scalar(min thr=10*m_a, accum add).

Sharding: atoms sharded across the 8 cores (672 each, padded to 768);
frames replicated. Each core emits one partial scalar; the host sums
the 8 partials (the gather/unshard step).

Layouts: frame f = 24*p + t (partition p, block t); slabs are
row-major in the feature index r with the block index t innermost
(unit stride) so DVE ops hit the packed 2x bf16 mode. Feature slabs
are transposed to [34, entity] via PE transposes (3 blocks packed per
transpose -> [102, 128] in PSUM) + strided DVE copies.
"""

import numpy as np

import concourse.bacc as bacc
import concourse.bass as bass
import concourse.tile as tile
from concourse import mybir
from concourse.bass_utils import run_bass_kernel_spmd

F32 = mybir.dt.float32
BF16 = mybir.dt.bfloat16
AX = mybir.AxisListType
OP = mybir.AluOpType
ACTF = mybir.ActivationFunctionType

NCORES = 8
NF = 3072          # frames (N*F)
TFB = 24           # frame blocks per partition (f = 24*p + t)
NA = 5376          # atoms (N*A)
NAS = NA // NCORES  # 672 atoms per core
NAPAD = 768        # padded per-core atoms
TAB = 6            # atom blocks per partition (a = 6*p + t)
K = 34             # feature dim
KS = 64            # stored feature rows (zero-padded, 128/KS-aligned for
                   # the DMA xbar transpose + 32-aligned strip copies)
CH = 1536          # frame chunk (PSUM cols) per ACT/DVE op
NCH = NF // CH     # 2 chunks
MMN = 512          # matmul moving free dim
X_FUSED = 12       # chunks (of 12) whose clamp+reduce is fused on DVE at 1x
                   # (measured cheapest reduction: ACT Copy+accum ~2.4us/chunk
                   # and PE ones-matmuls both regressed).
EPS = 1e-4
EPS_EFF = EPS      # sqrt(neg)=NaN is filtered by the DVE min (min(NaN,thr)=thr,
                   # verified on HW); rare tiny-d2 pairs hitting that path
                   # contribute ~1e-4 relative error.
CLAMP = 10.0
ZSCALE = 10.0
CNORM = float(1.0 / (ZSCALE * (3072.0 + EPS)))


def _bc(ap, dim, n):
    """Broadcast AP along a new axis at position `dim` (stride-0), n copies."""
    return ap.unsqueeze(dim).to_broadcast(
        tuple(ap.shape[:dim]) + (n,) + tuple(ap.shape[dim:])
    )


def build_nc():
    nc = bacc.Bacc(None)

    # inputs consolidated into two params -> two DMAs (queue latency is
    # ~2us per dma_start; nine separate loads stalled the feature ops)
    FRW = 2 * 9 * TFB + 2 * 3 * TFB          # pr|tr|pt|tt = 576
    ATW = 2 * 3 * TAB + TAB + NA // 128 + 128  # pp|tp|am|amf|ident = 212
    fr_d = nc.declare_dram_parameter("fr", [128, FRW], F32, isOutput=False)
    at_d = nc.declare_dram_parameter("at", [128, ATW], F32, isOutput=False)
    out_d = nc.declare_dram_parameter("out", [1, 2], F32, isOutput=True)

    with tile.TileContext(nc) as tc:
        with (
            tc.tile_pool(name="consts", bufs=1) as consts,
            tc.tile_pool(name="feat", bufs=1) as feat,
            tc.tile_pool(name="psum", bufs=2, space="PSUM") as psum_pool,
            tc.tile_pool(name="sbuf_s", bufs=3) as sbuf_s,
        ):
            # ---------------- input DMAs (two consolidated loads) --------
            frs = consts.tile([128, FRW], F32)
            ats = consts.tile([128, ATW], F32)
            nc.sync.dma_start(out=frs[:], in_=fr_d[:])
            nc.scalar.dma_start(out=ats[:], in_=at_d[:])
            fr_ap = frs[:]
            pRs = fr_ap[:, 0:216]
            tRs = fr_ap[:, 216:432]
            pts = fr_ap[:, 432:504]
            tts = fr_ap[:, 504:576]
            at_ap = ats[:]
            pps = at_ap[:, 0:18]
            tps = at_ap[:, 18:36]
            ams = at_ap[:, 36:42]
            amf = at_ap[:, 42:84]
            identf = at_ap[:, 84:212]
            identity = consts.tile([128, 128], BF16)
            nc.vector.tensor_copy(identity[:], identf)
            pRb, tRb, ptb, ttb, ppb, tpb = pRs, tRs, pts, tts, pps, tps

            # ------------- frame features (fp32, two t-halves) ----------
            # Slab col = KS*t + r. Computed per t-half so the first half's
            # transposes + main-loop chunk ch=0 overlap the second half.
            Wslab = feat.tile([128, KS * TFB], F32)
            nc.vector.memset(Wslab[:], 0.0)
            TH = TFB // 2
            mul0 = feat.tile([128, 9 * TH], F32)
            mul1 = feat.tile([128, 9 * TH], F32)
            mul2 = feat.tile([128, 9 * TH], F32)
            tmp3b = feat.tile([128, 3 * TH], F32)
            tmp3 = feat.tile([128, 3 * TFB], F32)
            tmp1 = feat.tile([128, TFB], F32)
            Ppt = feat.tile([128, 3 * TFB], F32)
            Mtt = feat.tile([128, 3 * TFB], F32)
            Ttt = feat.tile([128, 3 * TFB], F32)
            Mtp = feat.tile([128, 3 * TFB], F32)

            W4a = Wslab[:].rearrange("p (t r) -> p r t", r=KS)         # [128,64,24]
            R4a = pRb.rearrange("p (c t) -> p c t", c=9)
            T4a = tRb.rearrange("p (c t) -> p c t", c=9)
            pt3a = ptb.rearrange("p (c t) -> p c t", c=3)           # [128,3,24]
            tt3a = ttb.rearrange("p (c t) -> p c t", c=3)
            m0v = mul0[:].rearrange("p (i j t) -> p i j t", i=3, j=3)  # contig
            m1v = mul1[:].rearrange("p (i j t) -> p i j t", i=3, j=3)
            m2v = mul2[:].rearrange("p (i j t) -> p i j t", i=3, j=3)
            t3bv = tmp3b[:].rearrange("p (c t) -> p c t", c=3)

            def frame_feats(lo, hi):
                W4 = W4a[:, :, lo:hi]
                R4 = R4a[:, :, lo:hi].rearrange("p (i k) t -> p i k t", i=3)
                T4 = T4a[:, :, lo:hi].rearrange("p (i k) t -> p i k t", i=3)
                pt3 = pt3a[:, :, lo:hi]
                tt3 = tt3a[:, :, lo:hi]

                def gram(out4, A4, B4):
                    # 3 muls to contiguous temps, adds; only the last add
                    # writes the (KS-strided) slab rows.
                    a = lambda k: _bc(A4[:, :, k, :], 2, 3)
                    b = lambda k: _bc(B4[:, :, k, :], 1, 3)
                    nc.vector.tensor_mul(m0v, a(0), b(0))
                    nc.vector.tensor_mul(m1v, a(1), b(1))
                    nc.vector.tensor_mul(m2v, a(2), b(2))
                    nc.vector.tensor_add(m0v, m0v, m1v)
                    nc.vector.tensor_add(out4, m0v, m2v)

                Pv = W4[:, 0:9, :].rearrange("p (i j) t -> p i j t", i=3)
                Tv = W4[:, 9:18, :].rearrange("p (i j) t -> p i j t", i=3)
                Mv = W4[:, 18:27, :].rearrange("p (i j) t -> p i j t", i=3)
                gram(Pv, R4, R4)
                gram(Tv, T4, T4)
                gram(Mv, R4, T4)

                mjit = mul0[:].rearrange("p (j i t) -> p j i t", j=3, i=3)

                def matvec(out3, Q, vec3, transpose=False):
                    # one wide mul over (j,i,t), then two adds
                    qv = Q.transpose([0, 2, 1, 3]) if not transpose else Q
                    mj = mjit[:, :, :, 0:vec3.shape[2]]
                    nc.vector.tensor_mul(mj, qv, _bc(vec3, 2, 3))
                    nc.vector.tensor_add(t3bv, mj[:, 0, :, :], mj[:, 1, :, :])
                    nc.vector.tensor_add(out3, t3bv, mj[:, 2, :, :])

                Ppt3 = Ppt[:].rearrange("p (c t) -> p c t", c=3)[:, :, lo:hi]
                Mtt3 = Mtt[:].rearrange("p (c t) -> p c t", c=3)[:, :, lo:hi]
                Ttt3 = Ttt[:].rearrange("p (c t) -> p c t", c=3)[:, :, lo:hi]
                Mtp3 = Mtp[:].rearrange("p (c t) -> p c t", c=3)[:, :, lo:hi]
                matvec(Ppt3, Pv, pt3)
                matvec(Mtt3, Mv, tt3)
                matvec(Ttt3, Tv, tt3)
                matvec(Mtp3, Mv, pt3, transpose=True)  # M^T pt

                tmp3v = tmp3[:].rearrange("p (c t) -> p c t", c=3)[:, :, lo:hi]
                nc.vector.tensor_sub(tmp3v, Mtt3, Ppt3)
                nc.vector.tensor_scalar_mul(W4[:, 27:30, :], tmp3v, 2.0)
                nc.vector.tensor_sub(tmp3v, Mtp3, Ttt3)
                nc.vector.tensor_scalar_mul(W4[:, 30:33, :], tmp3v, 2.0)

                # cf row 33: pt.(Ppt - 2*Mtt) + tt.Ttt
                cfb = W4[:, 33, :]
                t1b = tmp1[:, lo:hi]
                nc.vector.tensor_sub(tmp3v, Ppt3, Mtt3)
                nc.vector.tensor_sub(tmp3v, tmp3v, Mtt3)
                # dot products via one wide mul each, then pairwise adds
                pd = t3bv  # [128,3,TH]
                nc.vector.tensor_mul(pd, tmp3v, pt3)
                nc.vector.tensor_add(t1b, pd[:, 0, :], pd[:, 1, :])
                nc.vector.tensor_add(cfb, t1b, pd[:, 2, :])
                nc.vector.tensor_mul(pd, Ttt3, tt3)
                nc.vector.tensor_add(t1b, pd[:, 0, :], pd[:, 1, :])
                nc.vector.tensor_add(t1b, t1b, pd[:, 2, :])
                nc.vector.tensor_add(cfb, cfb, t1b)

            # ------------- atom features (sharded) ----------------------
            Zslab = feat.tile([128, KS * TAB], F32)
            nc.vector.memset(Zslab[:], 0.0)
            Z4 = Zslab[:].rearrange("p (t r) -> p r t", r=KS)          # [128,34,6]
            pp3 = ppb.rearrange("p (c t) -> p c t", c=3)           # [128,3,6]
            tp3 = tpb.rearrange("p (c t) -> p c t", c=3)
            n2pp = feat.tile([128, 3 * TAB], F32)
            nc.vector.tensor_scalar_mul(n2pp[:], ppb, -2.0)
            n2pp3 = n2pp[:].rearrange("p (c t) -> p c t", c=3)

            Zpp = Z4[:, 0:9, :].rearrange("p (i j) t -> p i j t", i=3)
            Ztp = Z4[:, 9:18, :].rearrange("p (i j) t -> p i j t", i=3)
            Zx = Z4[:, 18:27, :].rearrange("p (i j) t -> p i j t", i=3)
            nc.vector.tensor_mul(Zpp, _bc(pp3, 2, 3), _bc(pp3, 1, 3))
            nc.vector.tensor_mul(Ztp, _bc(tp3, 2, 3), _bc(tp3, 1, 3))
            nc.vector.tensor_mul(Zx, _bc(n2pp3, 2, 3), _bc(tp3, 1, 3))
            nc.vector.tensor_copy(Z4[:, 27:30, :], pp3)
            nc.vector.tensor_copy(Z4[:, 30:33, :], tp3)
            nc.vector.memset(Z4[:, 33, :], 1.0)

            # mask-derived per-partition vectors (fp32)
            scale_v = consts.tile([128, TAB], F32)   # m^2
            bias_v = consts.tile([128, TAB], F32)    # m^2 * eps_eff
            thr_v = consts.tile([128, TAB], F32)     # 10 * m
            nc.vector.tensor_mul(scale_v[:], ams, ams)
            nc.vector.tensor_scalar_mul(bias_v[:], scale_v[:], EPS_EFF)
            nc.vector.tensor_scalar_mul(thr_v[:], ams, CLAMP)

            # ------------- transposes (PE, 128-col groups = 2 blocks) ---
            Wslab_b = feat.tile([128, KS * TFB], BF16)
            Zslab_b = feat.tile([128, KS * TAB], BF16)
            nc.vector.tensor_copy(Zslab_b[:], Zslab[:])
            NGW = KS * TFB // 128   # 12 groups
            NGZ = KS * TAB // 128   # 3 groups
            WT = consts.tile([KS, NF], BF16)
            ZT = consts.tile([KS, NAPAD], BF16)
            WT5 = WT[:].rearrange("q (g s c) -> q g s c", g=NGW, s=2)
            ZT5 = ZT[:].rearrange("q (g s c) -> q g s c", g=NGZ, s=2)

            def w_transpose_half(half):
                # cast this half of the slab, then 6 groups via PE
                HC = KS * TFB // 2
                nc.vector.tensor_copy(
                    Wslab_b[:, HC * half:HC * (half + 1)],
                    Wslab[:, HC * half:HC * (half + 1)])
                for q in range(3):
                    pst = psum_pool.tile([128, 512], BF16, tag="tp")
                    for u in range(2):
                        g = 6 * half + 2 * q + u
                        nc.tensor.transpose(
                            pst[:, 128 * u:128 * (u + 1)],
                            Wslab_b[:, 128 * g:128 * (g + 1)],
                            identity[:])
                    pst3 = pst[:, 0:256].rearrange("q (u c) -> q u c", c=128)
                    for s in range(2):
                        nc.vector.tensor_copy(
                            WT5[:, 6 * half + 2 * q:6 * half + 2 * q + 2, s, :],
                            pst3[64 * s:64 * (s + 1), :, :])

            pstz = psum_pool.tile([128, 512], BF16, tag="tp")
            for g in range(NGZ):
                nc.tensor.transpose(
                    pstz[:, 128 * g:128 * (g + 1)],
                    Zslab_b[:, 128 * g:128 * (g + 1)],
                    identity[:])
            pstz3 = pstz[:, 0:128 * NGZ].rearrange("q (u c) -> q u c", c=128)
            for s in range(2):
                nc.vector.tensor_copy(
                    ZT5[:, :, s, :], pstz3[64 * s:64 * (s + 1), :, :])


            frame_feats(0, TFB // 2)
            w_transpose_half(0)
            frame_feats(TFB // 2, TFB)
            w_transpose_half(1)

            # ------------- main loop ------------------------------------
            colacc = consts.tile([128, TAB * NCH], F32)
            scratch = consts.tile([128, CH], BF16)
            ones_b = consts.tile([128, 128], BF16)
            nc.vector.memset(ones_b[:], 1.0)

            pe_chunks = []
            idx = 0
            first_red = [True]
            for ch in range(NCH):
                for a in range(TAB):
                    zt = ZT[:, 128 * a:128 * (a + 1)]
                    ps = psum_pool.tile([128, CH], F32, tag="main")
                    for m in range(CH // MMN):
                        col = ch * CH + m * MMN
                        nc.tensor.matmul(
                            ps[:, m * MMN:(m + 1) * MMN],
                            zt,
                            WT[:, col:col + MMN],
                        )
                    s = sbuf_s.tile([128, CH], BF16)
                    nc.scalar.activation(
                        out=s[:],
                        in_=ps[:],
                        func=ACTF.Sqrt,
                        bias=bias_v[:, a:a + 1],
                        scale=scale_v[:, a:a + 1],
                    )
                    if (idx * X_FUSED) % 12 < X_FUSED and X_FUSED > 0:
                        nc.vector.tensor_scalar(
                            out=scratch[:],
                            in0=s[:],
                            scalar1=thr_v[:, a:a + 1],
                            scalar2=None,
                            op0=OP.min,
                            op1=OP.add,
                            accum_out=colacc[:, idx:idx + 1],
                        )
                    else:
                        # min at 4x on DVE (also filters sqrt-NaNs), then
                        # ScalarE sums the clamped tile via Copy+accum_out.
                        d = sbuf_s.tile([128, CH], BF16, tag="dmin")
                        nc.vector.tensor_scalar(
                            out=d[:],
                            in0=s[:],
                            scalar1=thr_v[:, a:a + 1],
                            scalar2=None,
                            op0=OP.min,
                        )
                        nc.scalar.activation(
                            out=scratch[:],
                            in_=d[:],
                            func=ACTF.Copy,
                            accum_out=colacc[:, idx:idx + 1],
                        )
                    idx += 1

            # ------------- epilogue -------------------------------------
            Sc = consts.tile([128, 1], F32)
            Sc2 = consts.tile([128, 1], F32)
            Mc = consts.tile([128, 1], F32)
            nc.vector.reduce_sum(out=Sc[:], in_=colacc[:], axis=AX.X)
            nc.vector.reduce_sum(out=Mc[:], in_=amf, axis=AX.X)
            ones_f = consts.tile([128, 1], F32)
            nc.vector.memset(ones_f[:], 1.0)
            psfin = psum_pool.tile([1, 2], F32, tag="tp")
            nc.tensor.matmul(psfin[:, 0:1], Sc[:], ones_f[:])
            nc.tensor.matmul(psfin[:, 1:2], Mc[:], ones_f[:])
            t0 = consts.tile([1, 1], F32)
            t1 = consts.tile([1, 1], F32)
            res = consts.tile([1, 2], F32)
            nc.vector.tensor_scalar(
                out=t0[:], in0=psfin[0:1, 1:2], scalar1=EPS, scalar2=None, op0=OP.add
            )
            nc.vector.reciprocal(t1[:], t0[:])
            nc.vector.tensor_scalar(
                out=res[:, 0:1], in0=psfin[0:1, 0:1], scalar1=t1[0:1, 0:1],
                scalar2=CNORM, op0=OP.mult, op1=OP.mult,
            )
            nc.vector.tensor_copy(res[:, 1:2], t0[:])
            nc.sync.dma_start(out=out_d[:], in_=res[:])

    nc.compile()
    return nc


def prep_in_maps(inputs):
    """Full (unsharded) numpy inputs -> per-core input dicts.

    Component-major SBUF layouts: frame f = 24*p + t lives at partition p,
    block t; a [*, C]-component tensor becomes [128, C*TFB] with column
    c*TFB + t. Atoms: a = 6*p + t, padded 672 -> 768 with zeros.
    """
    f32 = np.float32

    def fr(x, comps):
        return np.ascontiguousarray(
            np.asarray(x, f32).reshape(128, TFB, comps).transpose(0, 2, 1)
        ).reshape(128, comps * TFB)

    def at(x, comps, c):
        buf = np.zeros((NAPAD, comps), f32)
        buf[:NAS] = np.asarray(x, f32).reshape(NA, comps)[c * NAS:(c + 1) * NAS]
        return np.ascontiguousarray(
            buf.reshape(128, TAB, comps).transpose(0, 2, 1)
        ).reshape(128, comps * TAB)

    pR = fr(inputs["predicted_frames_R"], 9)
    tR = fr(inputs["true_frames_R"], 9)
    pt = fr(inputs["predicted_frames_t"], 3)
    tt = fr(inputs["true_frames_t"], 3)
    am_flat = np.asarray(inputs["atom_mask"], f32).reshape(NA)
    amf = np.ascontiguousarray(am_flat).reshape(128, NA // 128)

    fr = np.ascontiguousarray(np.concatenate([pR, tR, pt, tt], axis=1))
    ident = np.eye(128, dtype=f32)
    in_maps = []
    for c in range(NCORES):
        amp = np.zeros((NAPAD,), f32)
        amp[:NAS] = am_flat[c * NAS:(c + 1) * NAS]
        atc = np.ascontiguousarray(np.concatenate([
            at(inputs["predicted_atom_positions"], 3, c),
            at(inputs["true_atom_positions"], 3, c),
            amp.reshape(128, TAB),
            amf,
            ident,
        ], axis=1))
        in_maps.append({"fr": fr, "at": atc})
    return in_maps


_NC_CACHE = None


def _get_nc():
    global _NC_CACHE
    if _NC_CACHE is None:
        _NC_CACHE = build_nc()
    return _NC_CACHE


def kernel(**inputs):
    nc = _get_nc()
    in_maps = prep_in_maps(inputs)
    r = run_bass_kernel_spmd(nc, in_maps, core_ids=list(range(NCORES)))
    total = np.float32(0.0)
    for i in range(NCORES):
        total += np.float32(r.results[i]["out"][0, 0])
    return np.array([total], dtype=np.float32)



# revision 9
# speedup vs baseline: 1.4519x; 1.4519x over previous
"""AllAtomFAPE loss kernel for Trainium2 (8 NeuronCores, SPMD).

Problem: b=1, N=384 res, F=8 frames/res -> NF=3072 frames; A=14 atoms/res
-> NA=5376 atoms. Output: scalar (shape (1,)) masked clamped FAPE.

Algorithm (K=28 Gram factorization, host-precomputed features):
  lp - lt = A_f x_a with A_f = [pR^T | -tR^T | -w_f] (3x7),
  w_f = pR^T pt - tR^T tt, x_a = [pp; tp; 1] (7,). So
    d2(f,a) = x_a^T G_f x_a,  G_f = A_f^T A_f (7x7 PSD)
  which is a K=28 dot product between frame features W[:,f] (the 28
  unique entries of G, off-diagonals doubled) and atom features Z[:,a]
  (the matching monomials of x_a). Both feature slabs are computed on
  the host in float64 and rounded to bf16; the device does only the
  O(NF*NA) pairwise work:
    PE   : d2 tile = W_blk^T @ Z  (bf16 matmul, f32 PSUM)
    DVE  : clamp(d2, 0, 100) in-place in PSUM (min commutes with sqrt:
           min(sqrt(x+eps),10) = sqrt(min(x,100-)+eps); the max(.,0)
           guards bf16-rounding-induced negative d2 from NaN-ing sqrt)
    ACT  : sqrt(x + eps) with per-partition accumulate (accum_out)
  Epilogue: reduce accum columns, ones-matmul over partitions, DMA one
  f32 partial per core; host sums partials and normalizes.

Sharding: frames sharded across the 8 cores (384 each = 3 stationary
blocks of 128); atoms replicated. Atom features are packed as 4 groups
of 32 partitions (K=28 padded to 32) so the Z DMA uses all 128
partitions: group g occupies partitions [32g, 32g+28) and atom columns
[1536g, 1536g+1536) (group 3 holds 768 real cols + zeros). The W slab
is replicated into all 4 partition groups so every (block, group)
matmul has stationary and moving operands at the same partition offset.
"""

import numpy as np
import ml_dtypes

import concourse.bacc as bacc
import concourse.tile as tile
from concourse import mybir
from concourse.bass_utils import run_bass_kernel_spmd

F32 = mybir.dt.float32
BF16 = mybir.dt.bfloat16
AX = mybir.AxisListType
OP = mybir.AluOpType
ACTF = mybir.ActivationFunctionType

NCORES = 8
NF = 3072            # frames total
NFS = NF // NCORES   # 384 frames per core
NB = NFS // 128      # 3 stationary blocks per core
NA = 5376            # atoms (replicated on every core)
K = 28               # feature dim
# atom groups: (partition offset, zg column offset, columns). Matmul
# operands may only sit at partition offsets {0, 32, 64}, so the 4th
# 768-col group shares partition offset 0 at zg columns 1536+.
GROUPS = [(0, 0, 1536), (32, 0, 1536), (64, 0, 1536), (0, 1536, 768)]
GC = 2304            # zg tile columns (1536 + 768 on partition group 0)
EPS = 1e-4
CLAMP2 = 100.0       # clamp on d^2 (= 10.0 on d)
ZSCALE = 10.0


def build_nc():
    nc = bacc.Bacc(None)

    zg_d = nc.declare_dram_parameter("zg", [128, GC], BF16, isOutput=False)
    wr_d = nc.declare_dram_parameter("wr", [128, NFS], BF16, isOutput=False)
    out_d = nc.declare_dram_parameter("out", [1, 1], F32, isOutput=True)

    with tile.TileContext(nc) as tc:
        with (
            tc.tile_pool(name="consts", bufs=1) as consts,
            tc.tile_pool(name="psum", bufs=2, space="PSUM") as psum_pool,
            tc.tile_pool(name="acts", bufs=2) as acts,
        ):
            zg = consts.tile([128, GC], BF16)
            wr = consts.tile([128, NFS], BF16)
            nc.sync.dma_start(out=zg[:], in_=zg_d[:])
            nc.scalar.dma_start(out=wr[:], in_=wr_d[:])

            colacc = consts.tile([128, NB * len(GROUPS)], F32)
            bias_v = consts.tile([128, 1], F32)
            nc.vector.memset(bias_v[:], EPS)

            idx = 0
            for b in range(NB):
                for (poff, coff, cols) in GROUPS:
                    lhsT = wr[poff:poff + K, 128 * b:128 * (b + 1)]
                    ps = psum_pool.tile([128, 1536], F32, tag="main")
                    for off in range(0, cols, 512):
                        w = min(512, cols - off)
                        nc.tensor.matmul(
                            ps[:, off:off + w],
                            lhsT,
                            zg[poff:poff + K, coff + off:coff + off + w],
                        )
                    # clamp d2 to [0, 100] in-place (NaN-proofs the sqrt)
                    nc.vector.tensor_scalar(
                        out=ps[:, 0:cols],
                        in0=ps[:, 0:cols],
                        scalar1=0.0,
                        scalar2=CLAMP2,
                        op0=OP.max,
                        op1=OP.min,
                    )
                    s = acts.tile([128, 1536], BF16, tag="act")
                    nc.scalar.activation(
                        out=s[:, 0:cols],
                        in_=ps[:, 0:cols],
                        func=ACTF.Sqrt,
                        bias=bias_v[:, 0:1],
                        scale=1.0,
                        accum_out=colacc[:, idx:idx + 1],
                    )
                    idx += 1

            # ---- epilogue: one f32 partial (sum over all lanes) ----
            rsum = consts.tile([128, 1], F32)
            nc.vector.reduce_sum(out=rsum[:], in_=colacc[:], axis=AX.X)
            ones_f = consts.tile([128, 1], F32)
            nc.vector.memset(ones_f[:], 1.0)
            psfin = psum_pool.tile([1, 1], F32, tag="fin")
            nc.tensor.matmul(psfin[:], rsum[:], ones_f[:])
            res = consts.tile([1, 1], F32)
            nc.vector.tensor_copy(res[:], psfin[:])
            nc.sync.dma_start(out=out_d[:], in_=res[:])

    nc.compile()
    return nc


_SYM = [(0, 0), (0, 1), (0, 2), (1, 1), (1, 2), (2, 2)]


def _features(inputs):
    """Host-side K=28 feature slabs W [28, NF] and Z [28, NA] (float64)."""
    f8 = np.float64
    pR = np.asarray(inputs["predicted_frames_R"], f8).reshape(NF, 3, 3)
    tR = np.asarray(inputs["true_frames_R"], f8).reshape(NF, 3, 3)
    pt = np.asarray(inputs["predicted_frames_t"], f8).reshape(NF, 3)
    tt = np.asarray(inputs["true_frames_t"], f8).reshape(NF, 3)
    pp = np.asarray(inputs["predicted_atom_positions"], f8).reshape(NA, 3)
    tp = np.asarray(inputs["true_atom_positions"], f8).reshape(NA, 3)

    w = np.einsum("fki,fk->fi", pR, pt) - np.einsum("fki,fk->fi", tR, tt)
    A = np.concatenate(
        [pR.transpose(0, 2, 1), -tR.transpose(0, 2, 1), -w[:, :, None]], axis=2
    )  # (NF, 3, 7)
    G = np.einsum("fki,fkj->fij", A, A)  # (NF, 7, 7)

    W = np.empty((K, NF), f8)
    Z = np.empty((K, NA), f8)
    r = 0
    for (i, j) in _SYM:
        W[r] = G[:, i, j] * (1.0 if i == j else 2.0)
        Z[r] = pp[:, i] * pp[:, j]
        r += 1
    for (i, j) in _SYM:
        W[r] = G[:, 3 + i, 3 + j] * (1.0 if i == j else 2.0)
        Z[r] = tp[:, i] * tp[:, j]
        r += 1
    for i in range(3):
        for j in range(3):
            W[r] = 2.0 * G[:, i, 3 + j]
            Z[r] = pp[:, i] * tp[:, j]
            r += 1
    for i in range(3):
        W[r] = 2.0 * G[:, i, 6]
        Z[r] = pp[:, i]
        r += 1
    for i in range(3):
        W[r] = 2.0 * G[:, 3 + i, 6]
        Z[r] = tp[:, i]
        r += 1
    W[27] = G[:, 6, 6]
    Z[27] = 1.0
    return W, Z


def prep_in_maps(inputs):
    """Full numpy inputs -> per-core input dicts + host-side norm info."""
    bf = ml_dtypes.bfloat16
    W, Z = _features(inputs)

    am = np.asarray(inputs["atom_mask"], np.float64).reshape(NA)
    mask_zero = am <= 0.5
    n_zero = int(mask_zero.sum())
    if n_zero:
        Z[:, mask_zero] = 0.0

    # zg: [128, GC]; group i covers atom cols [1536i, 1536i+cols) and
    # lives at (partition offset poff, zg column offset coff)
    zg = np.zeros((128, GC), bf)
    for i, (poff, coff, cols) in enumerate(GROUPS):
        zg[poff:poff + K, coff:coff + cols] = (
            Z[:, 1536 * i:1536 * i + cols].astype(bf)
        )

    in_maps = []
    for c in range(NCORES):
        Wc = W[:, c * NFS:(c + 1) * NFS].astype(bf)  # [28, 384]
        wrc = np.zeros((128, NFS), bf)
        for poff in (0, 32, 64):
            wrc[poff:poff + K, :] = Wc
        in_maps.append({"zg": zg, "wr": wrc})

    norm = {
        "mask_sum": float(am.sum()),
        "pad_corr": 0.01 * float(NF) * n_zero,  # sqrt(eps) per zeroed pair
    }
    return in_maps, norm


_NC_CACHE = None


def _get_nc():
    global _NC_CACHE
    if _NC_CACHE is None:
        _NC_CACHE = build_nc()
    return _NC_CACHE


def kernel(**inputs):
    nc = _get_nc()
    in_maps, norm = prep_in_maps(inputs)
    r = run_bass_kernel_spmd(nc, in_maps, core_ids=list(range(NCORES)))
    total = 0.0
    for i in range(NCORES):
        total += float(r.results[i]["out"][0, 0])
    total -= norm["pad_corr"]
    res = total / (ZSCALE * (float(NF) + EPS) * (EPS + norm["mask_sum"]))
    return np.array([res], dtype=np.float32)


# revision 11
# speedup vs baseline: 1.8468x; 1.2720x over previous
"""AllAtomFAPE loss kernel for Trainium2 (8 NeuronCores, SPMD).

Problem: b=1, N=384 res, F=8 frames/res -> NF=3072 frames; A=14 atoms/res
-> NA=5376 atoms. Output: scalar (shape (1,)) masked clamped FAPE.

Algorithm (K=28 Gram factorization, host-precomputed features):
  lp - lt = A_f x_a with A_f = [pR^T | -tR^T | -w_f] (3x7),
  w_f = pR^T pt - tR^T tt, x_a = [pp; tp; 1] (7,). So
    d2(f,a) = x_a^T G_f x_a,  G_f = A_f^T A_f (7x7 PSD)
  which is a K=28 dot product between frame features W[:,f] (the 28
  unique entries of G, off-diagonals doubled) and atom features Z[:,a]
  (the matching monomials of x_a). Both feature slabs are computed on
  the host in float64 and rounded to bf16; the device does only the
  O(NF*NA) pairwise work:
    PE   : d2 tile = W_blk^T @ Z  (bf16 matmul, f32 PSUM)
    DVE  : clamp(d2, 0, 100) in-place in PSUM (min commutes with sqrt:
           min(sqrt(x+eps),10) = sqrt(min(x,100-)+eps); the max(.,0)
           guards bf16-rounding-induced negative d2 from NaN-ing sqrt)
    ACT  : sqrt(x + eps) with per-partition accumulate (accum_out)
  Epilogue: reduce accum columns, ones-matmul over partitions, DMA one
  f32 partial per core; host sums partials and normalizes.

Sharding: frames sharded across the 8 cores (384 each = 3 stationary
blocks of 128); atoms replicated. Atom features are packed as 4 groups
of 32 partitions (K=28 padded to 32) so the Z DMA uses all 128
partitions: group g occupies partitions [32g, 32g+28) and atom columns
[1536g, 1536g+1536) (group 3 holds 768 real cols + zeros). The W slab
is replicated into all 4 partition groups so every (block, group)
matmul has stationary and moving operands at the same partition offset.
"""

import numpy as np
import ml_dtypes

import concourse.bacc as bacc
import concourse.tile as tile
from concourse import mybir
from concourse.bass_utils import run_bass_kernel_spmd

F32 = mybir.dt.float32
BF16 = mybir.dt.bfloat16
AX = mybir.AxisListType
OP = mybir.AluOpType
ACTF = mybir.ActivationFunctionType

NCORES = 8
NF = 3072            # frames total
NFS = NF // NCORES   # 384 frames per core
NB = NFS // 128      # 3 stationary blocks per core
NA = 5376            # atoms (replicated on every core)
K = 28               # feature dim
# atom groups: (partition offset, zg column offset, columns). Matmul
# operands may only sit at partition offsets {0, 32, 64}, so the 4th
# 768-col group shares partition offset 0 at zg columns 1536+.
GROUPS = [(0, 0, 1536), (32, 0, 1536), (64, 0, 1536), (0, 1536, 768)]
GC = 2304            # zg tile columns (1536 + 768 on partition group 0)
EPS = 1e-4
CLAMP2 = 100.0       # clamp on d^2 (= 10.0 on d)
ZSCALE = 10.0


def build_nc():
    nc = bacc.Bacc(None)

    zg_d = nc.declare_dram_parameter("zg", [128, GC], BF16, isOutput=False)
    wr_d = nc.declare_dram_parameter("wr", [128, NFS], BF16, isOutput=False)
    out_d = nc.declare_dram_parameter("out", [1, 1], F32, isOutput=True)

    with tile.TileContext(nc) as tc:
        with (
            tc.tile_pool(name="consts", bufs=1) as consts,
            tc.tile_pool(name="psum", bufs=2, space="PSUM") as psum_pool,
            tc.tile_pool(name="dpool", bufs=2) as dpool,
            tc.tile_pool(name="acts", bufs=2) as acts,
        ):
            zg = consts.tile([128, GC], BF16)
            wr = consts.tile([128, NFS], BF16)
            # split zg so the first three groups land before group 3
            nc.sync.dma_start(out=zg[:, 0:1536], in_=zg_d[:, 0:1536])
            nc.scalar.dma_start(out=wr[:], in_=wr_d[:])
            nc.scalar.dma_start(out=zg[:, 1536:GC], in_=zg_d[:, 1536:GC])

            colacc = consts.tile([128, NB * 2], F32)
            bias_v = consts.tile([128, 1], F32)
            nc.vector.memset(bias_v[:], EPS)

            HB = 2688  # ACT half-block columns
            idx = 0
            for b in range(NB):
                db = dpool.tile([128, NA], BF16, tag="d")
                for gi, (poff, coff, cols) in enumerate(GROUPS):
                    lhsT = wr[poff:poff + K, 128 * b:128 * (b + 1)]
                    ps = psum_pool.tile([128, 1536], F32, tag="main")
                    for off in range(0, cols, 512):
                        w = min(512, cols - off)
                        nc.tensor.matmul(
                            ps[:, off:off + w],
                            lhsT,
                            zg[poff:poff + K, coff + off:coff + off + w],
                        )
                    # clamp d2 to [0, 100] -> bf16 (NaN-proofs the sqrt)
                    nc.vector.tensor_scalar(
                        out=db[:, 1536 * gi:1536 * gi + cols],
                        in0=ps[:, 0:cols],
                        scalar1=0.0,
                        scalar2=CLAMP2,
                        op0=OP.max,
                        op1=OP.min,
                    )
                    if gi % 2 == 1:
                        h = gi // 2
                        s = acts.tile([128, HB], BF16, tag="act")
                        nc.scalar.activation(
                            out=s[:],
                            in_=db[:, HB * h:HB * (h + 1)],
                            func=ACTF.Sqrt,
                            bias=bias_v[:, 0:1],
                            scale=1.0,
                            accum_out=colacc[:, idx:idx + 1],
                        )
                        idx += 1

            # ---- epilogue: one f32 partial (sum over all lanes) ----
            rsum = consts.tile([128, 1], F32)
            nc.vector.reduce_sum(out=rsum[:], in_=colacc[:], axis=AX.X)
            ones_f = consts.tile([128, 1], F32)
            nc.vector.memset(ones_f[:], 1.0)
            psfin = psum_pool.tile([1, 1], F32, tag="fin")
            nc.tensor.matmul(psfin[:], rsum[:], ones_f[:])
            res = consts.tile([1, 1], F32)
            nc.vector.tensor_copy(res[:], psfin[:])
            nc.sync.dma_start(out=out_d[:], in_=res[:])

    nc.compile()
    return nc


_SYM = [(0, 0), (0, 1), (0, 2), (1, 1), (1, 2), (2, 2)]


def _features(inputs):
    """Host-side K=28 feature slabs W [28, NF] and Z [28, NA] (float64)."""
    f8 = np.float64
    pR = np.asarray(inputs["predicted_frames_R"], f8).reshape(NF, 3, 3)
    tR = np.asarray(inputs["true_frames_R"], f8).reshape(NF, 3, 3)
    pt = np.asarray(inputs["predicted_frames_t"], f8).reshape(NF, 3)
    tt = np.asarray(inputs["true_frames_t"], f8).reshape(NF, 3)
    pp = np.asarray(inputs["predicted_atom_positions"], f8).reshape(NA, 3)
    tp = np.asarray(inputs["true_atom_positions"], f8).reshape(NA, 3)

    w = np.einsum("fki,fk->fi", pR, pt) - np.einsum("fki,fk->fi", tR, tt)
    A = np.concatenate(
        [pR.transpose(0, 2, 1), -tR.transpose(0, 2, 1), -w[:, :, None]], axis=2
    )  # (NF, 3, 7)
    G = np.einsum("fki,fkj->fij", A, A)  # (NF, 7, 7)

    W = np.empty((K, NF), f8)
    Z = np.empty((K, NA), f8)
    r = 0
    for (i, j) in _SYM:
        W[r] = G[:, i, j] * (1.0 if i == j else 2.0)
        Z[r] = pp[:, i] * pp[:, j]
        r += 1
    for (i, j) in _SYM:
        W[r] = G[:, 3 + i, 3 + j] * (1.0 if i == j else 2.0)
        Z[r] = tp[:, i] * tp[:, j]
        r += 1
    for i in range(3):
        for j in range(3):
            W[r] = 2.0 * G[:, i, 3 + j]
            Z[r] = pp[:, i] * tp[:, j]
            r += 1
    for i in range(3):
        W[r] = 2.0 * G[:, i, 6]
        Z[r] = pp[:, i]
        r += 1
    for i in range(3):
        W[r] = 2.0 * G[:, 3 + i, 6]
        Z[r] = tp[:, i]
        r += 1
    W[27] = G[:, 6, 6]
    Z[27] = 1.0
    return W, Z


def prep_in_maps(inputs):
    """Full numpy inputs -> per-core input dicts + host-side norm info."""
    bf = ml_dtypes.bfloat16
    W, Z = _features(inputs)

    am = np.asarray(inputs["atom_mask"], np.float64).reshape(NA)
    mask_zero = am <= 0.5
    n_zero = int(mask_zero.sum())
    if n_zero:
        Z[:, mask_zero] = 0.0

    # zg: [128, GC]; group i covers atom cols [1536i, 1536i+cols) and
    # lives at (partition offset poff, zg column offset coff)
    zg = np.zeros((128, GC), bf)
    for i, (poff, coff, cols) in enumerate(GROUPS):
        zg[poff:poff + K, coff:coff + cols] = (
            Z[:, 1536 * i:1536 * i + cols].astype(bf)
        )

    in_maps = []
    for c in range(NCORES):
        Wc = W[:, c * NFS:(c + 1) * NFS].astype(bf)  # [28, 384]
        wrc = np.zeros((128, NFS), bf)
        for poff in (0, 32, 64):
            wrc[poff:poff + K, :] = Wc
        in_maps.append({"zg": zg, "wr": wrc})

    norm = {
        "mask_sum": float(am.sum()),
        "pad_corr": 0.01 * float(NF) * n_zero,  # sqrt(eps) per zeroed pair
    }
    return in_maps, norm


_NC_CACHE = None


def _get_nc():
    global _NC_CACHE
    if _NC_CACHE is None:
        _NC_CACHE = build_nc()
    return _NC_CACHE


def kernel(**inputs):
    nc = _get_nc()
    in_maps, norm = prep_in_maps(inputs)
    r = run_bass_kernel_spmd(nc, in_maps, core_ids=list(range(NCORES)))
    total = 0.0
    for i in range(NCORES):
        total += float(r.results[i]["out"][0, 0])
    total -= norm["pad_corr"]
    res = total / (ZSCALE * (float(NF) + EPS) * (EPS + norm["mask_sum"]))
    return np.array([res], dtype=np.float32)


# revision 14
# speedup vs baseline: 1.8552x; 1.0046x over previous
"""AllAtomFAPE loss kernel for Trainium2 (8 NeuronCores, SPMD).

Problem: b=1, N=384 res, F=8 frames/res -> NF=3072 frames; A=14 atoms/res
-> NA=5376 atoms. Output: scalar (shape (1,)) masked clamped FAPE.

Algorithm (K=28 Gram factorization, host-precomputed features):
  lp - lt = A_f x_a with A_f = [pR^T | -tR^T | -w_f] (3x7),
  w_f = pR^T pt - tR^T tt, x_a = [pp; tp; 1] (7,). So
    d2(f,a) = x_a^T G_f x_a,  G_f = A_f^T A_f (7x7 PSD)
  which is a K=28 dot product between frame features W[:,f] (the 28
  unique entries of G, off-diagonals doubled) and atom features Z[:,a]
  (the matching monomials of x_a). Both feature slabs are computed on
  the host in float64 and rounded to bf16; the device does only the
  O(NF*NA) pairwise work:
    PE   : d2 tile = W_blk^T @ Z  (bf16 matmul, f32 PSUM)
    DVE  : clamp(d2, 0, 100) in-place in PSUM (min commutes with sqrt:
           min(sqrt(x+eps),10) = sqrt(min(x,100-)+eps); the max(.,0)
           guards bf16-rounding-induced negative d2 from NaN-ing sqrt)
    ACT  : sqrt(x + eps) with per-partition accumulate (accum_out)
  Epilogue: reduce accum columns, ones-matmul over partitions, DMA one
  f32 partial per core; host sums partials and normalizes.

Sharding: frames sharded across the 8 cores (384 each = 3 stationary
blocks of 128); atoms replicated. Atom features are packed as 4 groups
of 32 partitions (K=28 padded to 32) so the Z DMA uses all 128
partitions: group g occupies partitions [32g, 32g+28) and atom columns
[1536g, 1536g+1536) (group 3 holds 768 real cols + zeros). The W slab
is replicated into all 4 partition groups so every (block, group)
matmul has stationary and moving operands at the same partition offset.
"""

import numpy as np
import ml_dtypes

import concourse.bacc as bacc
import concourse.tile as tile
from concourse import mybir
from concourse.bass_utils import run_bass_kernel_spmd

F32 = mybir.dt.float32
BF16 = mybir.dt.bfloat16
AX = mybir.AxisListType
OP = mybir.AluOpType
ACTF = mybir.ActivationFunctionType

NCORES = 8
NF = 3072            # frames total
NFS = NF // NCORES   # 384 frames per core
NB = NFS // 128      # 3 stationary blocks per core
NA = 5376            # atoms (replicated on every core)
K = 28               # feature dim
# atom groups: (partition offset, zg column offset, columns). Matmul
# operands may only sit at partition offsets {0, 32, 64}, so the 4th
# 768-col group shares partition offset 0 at zg columns 1536+.
GROUPS = [(0, 0, 1536), (32, 0, 1536), (64, 0, 1536), (0, 1536, 768)]
GC = 2304            # zg tile columns (1536 + 768 on partition group 0)
EPS = 1e-4
CLAMP2 = 100.0       # clamp on d^2 (= 10.0 on d)
ZSCALE = 10.0


def build_nc():
    nc = bacc.Bacc(None)

    zg_d = nc.declare_dram_parameter("zg", [128, GC], BF16, isOutput=False)
    wr_d = nc.declare_dram_parameter("wr", [128, NFS], BF16, isOutput=False)
    out_d = nc.declare_dram_parameter("out", [1, 1], F32, isOutput=True)

    with tile.TileContext(nc) as tc:
        with (
            tc.tile_pool(name="consts", bufs=1) as consts,
            tc.tile_pool(name="psum", bufs=2, space="PSUM") as psum_pool,
            tc.tile_pool(name="dpool", bufs=3) as dpool,
            tc.tile_pool(name="acts", bufs=2) as acts,
        ):
            zg = consts.tile([128, GC], BF16)
            wr = consts.tile([128, NFS], BF16)
            # DMAs stay off the Scalar queue so ACT table loads overlap them;
            # zg is split so the first three groups land before group 3
            nc.sync.dma_start(out=zg[:, 0:1536], in_=zg_d[:, 0:1536])
            nc.gpsimd.dma_start(out=wr[:], in_=wr_d[:])
            nc.gpsimd.dma_start(out=zg[:, 1536:GC], in_=zg_d[:, 1536:GC])

            colacc = consts.tile([128, 2 * (NB - 1) + len(GROUPS)], F32)
            bias_v = consts.tile([128, 1], F32)
            nc.vector.memset(bias_v[:], EPS)

            # ACT spans: half-blocks, except the last block runs per-tile
            # spans so the pipeline tail is short. Span (gi, start, width)
            # is emitted after the DVE clamp of group gi.
            HALF = [(1, 0, 2688), (3, 2688, 2688)]
            FINE = [(0, 0, 1536), (1, 1536, 1536), (2, 3072, 1536),
                    (3, 4608, 768)]
            idx = 0
            for b in range(NB):
                db = dpool.tile([128, NA], BF16, tag="d")
                spans = FINE if b == NB - 1 else HALF
                for gi, (poff, coff, cols) in enumerate(GROUPS):
                    lhsT = wr[poff:poff + K, 128 * b:128 * (b + 1)]
                    ps = psum_pool.tile([128, 1536], F32, tag="main")
                    for off in range(0, cols, 512):
                        w = min(512, cols - off)
                        nc.tensor.matmul(
                            ps[:, off:off + w],
                            lhsT,
                            zg[poff:poff + K, coff + off:coff + off + w],
                        )
                    # clamp d2 to [0, 100] -> bf16 (NaN-proofs the sqrt)
                    nc.vector.tensor_scalar(
                        out=db[:, 1536 * gi:1536 * gi + cols],
                        in0=ps[:, 0:cols],
                        scalar1=0.0,
                        scalar2=CLAMP2,
                        op0=OP.max,
                        op1=OP.min,
                    )
                    for (sg, start, width) in spans:
                        if sg != gi:
                            continue
                        s = acts.tile([128, 2688], BF16, tag="act")
                        nc.scalar.activation(
                            out=s[:, 0:width],
                            in_=db[:, start:start + width],
                            func=ACTF.Sqrt,
                            bias=bias_v[:, 0:1],
                            scale=1.0,
                            accum_out=colacc[:, idx:idx + 1],
                        )
                        idx += 1

            # ---- epilogue: one f32 partial (sum over all lanes) ----
            rsum = consts.tile([128, 1], F32)
            nc.vector.reduce_sum(out=rsum[:], in_=colacc[:], axis=AX.X)
            ones_f = consts.tile([128, 1], F32)
            nc.vector.memset(ones_f[:], 1.0)
            psfin = psum_pool.tile([1, 1], F32, tag="fin")
            nc.tensor.matmul(psfin[:], rsum[:], ones_f[:])
            res = consts.tile([1, 1], F32)
            nc.vector.tensor_copy(res[:], psfin[:])
            nc.sync.dma_start(out=out_d[:], in_=res[:])

    nc.compile()
    return nc


_SYM = [(0, 0), (0, 1), (0, 2), (1, 1), (1, 2), (2, 2)]


def _features(inputs):
    """Host-side K=28 feature slabs W [28, NF] and Z [28, NA] (float64)."""
    f8 = np.float64
    pR = np.asarray(inputs["predicted_frames_R"], f8).reshape(NF, 3, 3)
    tR = np.asarray(inputs["true_frames_R"], f8).reshape(NF, 3, 3)
    pt = np.asarray(inputs["predicted_frames_t"], f8).reshape(NF, 3)
    tt = np.asarray(inputs["true_frames_t"], f8).reshape(NF, 3)
    pp = np.asarray(inputs["predicted_atom_positions"], f8).reshape(NA, 3)
    tp = np.asarray(inputs["true_atom_positions"], f8).reshape(NA, 3)

    w = np.einsum("fki,fk->fi", pR, pt) - np.einsum("fki,fk->fi", tR, tt)
    A = np.concatenate(
        [pR.transpose(0, 2, 1), -tR.transpose(0, 2, 1), -w[:, :, None]], axis=2
    )  # (NF, 3, 7)
    G = np.einsum("fki,fkj->fij", A, A)  # (NF, 7, 7)

    W = np.empty((K, NF), f8)
    Z = np.empty((K, NA), f8)
    r = 0
    for (i, j) in _SYM:
        W[r] = G[:, i, j] * (1.0 if i == j else 2.0)
        Z[r] = pp[:, i] * pp[:, j]
        r += 1
    for (i, j) in _SYM:
        W[r] = G[:, 3 + i, 3 + j] * (1.0 if i == j else 2.0)
        Z[r] = tp[:, i] * tp[:, j]
        r += 1
    for i in range(3):
        for j in range(3):
            W[r] = 2.0 * G[:, i, 3 + j]
            Z[r] = pp[:, i] * tp[:, j]
            r += 1
    for i in range(3):
        W[r] = 2.0 * G[:, i, 6]
        Z[r] = pp[:, i]
        r += 1
    for i in range(3):
        W[r] = 2.0 * G[:, 3 + i, 6]
        Z[r] = tp[:, i]
        r += 1
    W[27] = G[:, 6, 6]
    Z[27] = 1.0
    return W, Z


def prep_in_maps(inputs):
    """Full numpy inputs -> per-core input dicts + host-side norm info."""
    bf = ml_dtypes.bfloat16
    W, Z = _features(inputs)

    am = np.asarray(inputs["atom_mask"], np.float64).reshape(NA)
    mask_zero = am <= 0.5
    n_zero = int(mask_zero.sum())
    if n_zero:
        Z[:, mask_zero] = 0.0

    # zg: [128, GC]; group i covers atom cols [1536i, 1536i+cols) and
    # lives at (partition offset poff, zg column offset coff)
    zg = np.zeros((128, GC), bf)
    for i, (poff, coff, cols) in enumerate(GROUPS):
        zg[poff:poff + K, coff:coff + cols] = (
            Z[:, 1536 * i:1536 * i + cols].astype(bf)
        )

    in_maps = []
    for c in range(NCORES):
        Wc = W[:, c * NFS:(c + 1) * NFS].astype(bf)  # [28, 384]
        wrc = np.zeros((128, NFS), bf)
        for poff in (0, 32, 64):
            wrc[poff:poff + K, :] = Wc
        in_maps.append({"zg": zg, "wr": wrc})

    norm = {
        "mask_sum": float(am.sum()),
        "pad_corr": 0.01 * float(NF) * n_zero,  # sqrt(eps) per zeroed pair
    }
    return in_maps, norm


_NC_CACHE = None


def _get_nc():
    global _NC_CACHE
    if _NC_CACHE is None:
        _NC_CACHE = build_nc()
    return _NC_CACHE


def kernel(**inputs):
    nc = _get_nc()
    in_maps, norm = prep_in_maps(inputs)
    r = run_bass_kernel_spmd(nc, in_maps, core_ids=list(range(NCORES)))
    total = 0.0
    for i in range(NCORES):
        total += float(r.results[i]["out"][0, 0])
    total -= norm["pad_corr"]
    res = total / (ZSCALE * (float(NF) + EPS) * (EPS + norm["mask_sum"]))
    return np.array([res], dtype=np.float32)
